# revision 1
# baseline (speedup 1.0000x reference)
"""Trainium2 Bass kernel for nn_DecoderWithAttention.

2-layer GRU decoder with Bahdanau attention, 12 sequential timesteps.
Strategy: data-parallel over batch (64 -> 8 cores x 8), weights replicated.
Per core, matmuls are batch-major [8, *] (weights stream through the PE as
the moving operand, gate matmuls packed 4-wide with PE column tiling);
state is kept transposed [unit, batch] for use as matmul lhsT.  The
attention context (ws) is never materialized: its contribution to the
gates and output projection is folded in via encW = enc @ W_ws.T
(precomputed once on device) contracted with block-diagonal softmax
weights.
"""
import sys
sys.path.insert(0, '/opt/trn_rl_repo')
import numpy as np

B, DEC, F = 64, 12, 32
L, H = 2, 512
E, T = 96, 4
N_CORES = 8
BS = B // N_CORES  # 8 batches per core

_COMPILED = {}


def _f32(x):
    return np.ascontiguousarray(x, dtype=np.float32)


def _bf16(x):
    import ml_dtypes
    return np.ascontiguousarray(np.asarray(x, dtype=np.float32).astype(ml_dtypes.bfloat16))


def build_nc():
    import concourse.bass as bass
    import concourse.tile as tile
    from concourse import mybir
    from concourse.vector_clock import ScopedClock

    f32 = mybir.dt.float32
    bf16 = mybir.dt.bfloat16
    AF = mybir.ActivationFunctionType

    # --- patch: the TileContext exit drain gets >1 sem wait, which this
    # walrus rejects ("Too many sync wait commands"); split into
    # single-wait drains. ---
    def patched_drain(self, tick_clock, wait_clock):
        nc = self.nc
        drain_inst = nc.sync.drain()
        wait_clock.add_sem_waits(
            drain_inst.ins, ScopedClock({None: tick_clock.global_clock}))
        si = drain_inst.ins.sync_info
        waits = list(si.on_wait or [])
        if len(waits) > 1:
            SyncInfo = type(si)
            drain_inst.ins.sync_info = SyncInfo(
                on_wait=[waits[0]], on_update=list(si.on_update or []))
            for w in waits[1:]:
                d2 = nc.sync.drain()
                d2.ins.sync_info = SyncInfo(on_wait=[w], on_update=[])
        nc.all_engine_barrier()
        assert self.sems is not None
        popped = nc._tile_sem_poison_stack.pop()
        assert popped is self._sem_poison
        nc.clear_and_free_semaphores(list(self.sems.allocated().values()))
        nc.all_engine_barrier()

    tile.TileContext._drain_and_barrier = patched_drain

    nc = bass.Bass()

    def P(name, shape, dt=f32):
        return nc.declare_dram_parameter(name, list(shape), dt, isOutput=False)

    # per-core inputs
    inputsT_e = P("inputsT", [32, DEC, BS], bf16)
    h0T_e = P("h0T", [128, 4, BS], bf16)
    h1T_e = P("h1T", [128, 4, BS], bf16)
    h0m_e = P("h0m", [BS, H])
    h1m_e = P("h1m", [BS, H])
    encT_e = P("encT", [128, 4, BS * E], bf16)  # [r, k, (b e)]
    # replicated weights
    waeT_e = P("waeT", [128, 4, H], bf16)
    battn_e = P("battn", [1, H], bf16)
    onesr_e = P("onesr", [1, BS * E], bf16)
    wahT_e = P("wahT", [128, 4, H], bf16)
    v_e = P("v", [128, 4, 4, BS], bf16)   # block-diag v for dense scores
    a0_e = P("a0", [128, 5, 2048], bf16)  # layer0 h/cur/bias part
    aw_e = P("aw", [128, 4, 1540], bf16)  # ws-part weights (for encW setup)
    a1_e = P("a1", [128, 9, 2048], bf16)
    wo_e = P("wo", [128, 5, T], bf16)     # h1'/cur/bias part of W_out
    ident_e = P("ident", [8, 8])
    x0t8_e = P("x0t8", [128, BS], bf16)   # xh0T chunk-4 init (cur0+ones)
    x1t8_e = P("x1t8", [128, BS], bf16)   # xh1T chunk-8 init (ones)
    out_e = nc.declare_dram_parameter("out", [BS, DEC, T], f32, isOutput=True)

    with tile.TileContext(nc) as tc:
        with tc.tile_pool(name="wts", bufs=1) as wts, \
             tc.tile_pool(name="state", bufs=1) as st, \
             tc.tile_pool(name="work", bufs=2) as wk, \
             tc.tile_pool(name="psg", bufs=2, space="PSUM") as psg, \
             tc.tile_pool(name="pss", bufs=6, space="PSUM") as pss:

            # ---- load everything into SBUF ----
            def load(pool, ext, shape, dt):
                t = pool.tile(list(shape), dt, tag=ext.name)
                nc.sync.dma_start(t[:], ext[:])
                return t

            ident = load(wts, ident_e, [8, 8], f32)
            waeT = load(wts, waeT_e, [128, 4, H], bf16)
            battn = load(wts, battn_e, [1, H], bf16)
            onesr = load(wts, onesr_e, [1, BS * E], bf16)
            encT = load(wts, encT_e, [128, 4, BS * E], bf16)
            wahT = load(wts, wahT_e, [128, 4, H], bf16)
            v_sb = load(wts, v_e, [128, 4, 4, BS], bf16)
            a0 = load(wts, a0_e, [128, 5, 2048], bf16)
            aw = load(wts, aw_e, [128, 4, 1540], bf16)
            a1 = load(wts, a1_e, [128, 9, 2048], bf16)
            wo = load(wts, wo_e, [128, 5, T], bf16)
            inT = load(wts, inputsT_e, [32, DEC, BS], bf16)

            # persistent state tiles
            xh0T = st.tile([128, 5, BS], bf16)   # [h0T(0:4) | cur/ones(4)]
            xh1T = st.tile([128, 9, BS], bf16)   # [h0'T(0:4) | h1T(4:8) | ones(8)]
            nc.sync.dma_start(xh0T[:, 0:4, :], h0T_e[:])
            nc.sync.dma_start(xh1T[:, 4:8, :], h1T_e[:])
            nc.sync.dma_start(xh0T[:, 4, :], x0t8_e[:])
            nc.sync.dma_start(xh1T[:, 8, :], x1t8_e[:])
            h0m = st.tile([BS, H], f32)
            h1m = st.tile([BS, H], f32)
            nc.sync.dma_start(h0m[:], h0m_e[:])
            nc.sync.dma_start(h1m[:], h1m_e[:])
            wd = st.tile([128, BS, BS], bf16)    # block-diag softmax weights
            nc.vector.memset(wd[:], 0.0)
            encP = st.tile([128, 4, BS * E], bf16)   # enc_projT (+b_attn)
            encW = st.tile([128, BS, 1540], bf16)    # enc @ [Wi_ws|Wo_ws].T
            nc.vector.memset(encW[:], 0.0)
            outb = st.tile([BS, DEC, T], f32)

            # ---- one-time: enc_projT = Wa_e @ encT + b_attn ----
            for m in range(4):
                for n2 in range(2):
                    ns = slice(n2 * 384, (n2 + 1) * 384)
                    pe = psg.tile([128, 384], f32, tag="g")
                    for k in range(4):
                        nc.tensor.matmul(
                            pe[:], waeT[:, k, m * 128:(m + 1) * 128],
                            encT[:, k, ns], start=(k == 0), stop=False)
                    nc.tensor.matmul(
                        pe[:], battn[0:1, m * 128:(m + 1) * 128],
                        onesr[0:1, ns], start=False, stop=True)
                    nc.vector.tensor_copy(encP[:, m, ns], pe[:])

            # ---- one-time: encW[b] = enc[b] @ [Wi*_ws | Wout_ws].T ----
            for b in range(BS):
                es = slice(b * E, (b + 1) * E)
                for n3 in range(3):
                    ns = slice(n3 * 512, (n3 + 1) * 512)
                    pe = psg.tile([128, 512], f32, tag="g")
                    for k in range(4):
                        nc.tensor.matmul(pe[0:E, :], encT[:, k, es],
                                         aw[:, k, ns],
                                         start=(k == 0), stop=(k == 3))
                    if (b + n3) % 2 == 0:
                        nc.vector.tensor_copy(encW[0:E, b, ns], pe[0:E, :])
                    else:
                        nc.scalar.activation(encW[0:E, b, ns], pe[0:E, :], AF.Copy)
                peo = pss.tile([E, T], f32, tag="s")
                for k in range(4):
                    nc.tensor.matmul(peo[:], encT[:, k, es], aw[:, k, 1536:1540],
                                     start=(k == 0), stop=(k == 3))
                nc.vector.tensor_copy(encW[0:E, b, 1536:1540], peo[:])

            def gru_layer(groups, pg):
                """gates matmuls (4-way col-tiled) + gate math -> returns
                (rt, zt) started; groups = per-colgroup list of (lhsT, rhs)."""
                for g, pairs in enumerate(groups):
                    for i, (lhsT, rhs) in enumerate(pairs):
                        nc.tensor.matmul(
                            pg[32 * g:32 * g + BS, :], lhsT, rhs,
                            start=(i == 0), stop=(i == len(pairs) - 1),
                            tile_position=(0, 32 * g))

            def gru_math(pg, hm):
                rt = wk.tile([BS, H], bf16, tag="rt")
                nc.scalar.activation(rt[:], pg[0:BS, :], AF.Sigmoid)
                zt = wk.tile([BS, H], bf16, tag="zt")
                nc.scalar.activation(zt[:], pg[32:32 + BS, :], AF.Sigmoid)
                m1 = wk.tile([BS, H], f32, tag="m1")
                nc.vector.tensor_mul(m1[:], rt[:], pg[96:96 + BS, :])
                tt = wk.tile([BS, H], f32, tag="tt")
                nc.vector.tensor_add(tt[:], m1[:], pg[64:64 + BS, :])
                nt = wk.tile([BS, H], f32, tag="nt")
                nc.scalar.activation(nt[:], tt[:], AF.Tanh)
                s = wk.tile([BS, H], f32, tag="s")
                nc.vector.tensor_sub(s[:], hm[:], nt[:])
                m2 = wk.tile([BS, H], f32, tag="m2")
                nc.vector.tensor_mul(m2[:], zt[:], s[:])
                nc.vector.tensor_add(hm[:], nt[:], m2[:])

            def transpose_to(src8, dsts):
                """src8 [8,512] fp32 sbuf -> each dst [128,4,8] bf16."""
                pt = pss.tile([128, 32], f32, tag="s")
                for j in range(4):
                    nc.tensor.transpose(pt[:, j * 8:(j + 1) * 8],
                                        src8[0:BS, j * 128:(j + 1) * 128],
                                        ident[:])
                for dst in dsts:
                    nc.vector.tensor_copy(
                        dst, pt[:].rearrange("p (j b) -> p j b", j=4))

            # ================= time loop =================
            for t in range(DEC):
                # --- qT = Wa_h @ h1 : [128,4,8] psum ---
                pq = pss.tile([128, 4, BS], f32, tag="s")
                for m in range(4):
                    for k in range(4):
                        nc.tensor.matmul(
                            pq[:, m, :], wahT[:, k, m * 128:(m + 1) * 128],
                            xh1T[:, 4 + k, :], start=(k == 0), stop=(k == 3))
                qb = wk.tile([128, 4, BS], bf16, tag="qb")
                nc.vector.tensor_copy(qb[:], pq[:])

                # --- energy = tanh(encP + qT) (bf16), chunked in 2 ---
                ea = wk.tile([128, 4, BS * E], bf16, tag="ea")
                en = wk.tile([128, 4, BS * E], bf16, tag="en")
                for hh in range(2):
                    ms = slice(2 * hh, 2 * hh + 2)
                    nc.vector.tensor_add(
                        ea[:, ms, :].rearrange("p m (b e) -> p m b e", b=BS),
                        encP[:, ms, :].rearrange("p m (b e) -> p m b e", b=BS),
                        qb[:, ms, :].unsqueeze(3).broadcast_to((128, 2, BS, E)))
                    nc.scalar.activation(en[:, ms, :], ea[:, ms, :], AF.Tanh)

                # --- scores, dense [8, 192] via block-diag v: batch pair p
                # contributes rows {2p, 2p+1}; scores[b] lands at row b,
                # cols (b%2)*96 (other half of the row is a harmless
                # duplicate of the partner batch). ---
                ps_s = pss.tile([BS, 2 * E], f32, tag="s")
                for k in range(4):
                    for p in range(4):
                        nc.tensor.matmul(
                            ps_s[:], v_sb[:, k, p, :],
                            en[:, k, 2 * p * E:(2 * p + 2) * E],
                            start=(p == 0 and k == 0), stop=(p == 3 and k == 3))

                # --- softmax over E (scores are small: skip max-sub) ---
                w_s = wk.tile([BS, 2 * E], f32, tag="ws_sm")
                zz = wk.tile([BS, 2], f32, tag="zz")
                for r in range(2):
                    cs = slice(r * E, (r + 1) * E)
                    nc.scalar.activation(w_s[:, cs], ps_s[:, cs], AF.Exp,
                                         accum_out=zz[:, r:r + 1])
                nc.vector.reciprocal(zz[:], zz[:])
                for r in range(2):
                    cs = slice(r * E, (r + 1) * E)
                    nc.vector.tensor_scalar_mul(w_s[:, cs], w_s[:, cs],
                                                zz[:, r:r + 1])

                # --- wT then block-diag wd; col b of transpose r=b%2 is
                # batch b's normalized weights ---
                pwT = pss.tile([128, 2 * BS], f32, tag="s")
                for r in range(2):
                    nc.tensor.transpose(pwT[0:E, r * BS:(r + 1) * BS],
                                        w_s[:, r * E:(r + 1) * E], ident[:])
                wTs = wk.tile([128, 2 * BS], f32, tag="wTs")
                nc.vector.tensor_copy(wTs[0:E, :], pwT[0:E, :])
                for b in range(BS):
                    nc.vector.tensor_copy(
                        wd[0:E, b, b:b + 1],
                        wTs[0:E, (b % 2) * BS + b:(b % 2) * BS + b + 1])

                # --- GRU layer 0 (ws folded in via wd x encW) ---
                wd_pairs = lambda cols: [(wd[:, b, :], encW[:, b, cols])
                                         for b in range(BS)]
                h_pairs0 = lambda cols, ks: [(xh0T[:, k, :], a0[:, k, cols])
                                             for k in ks]
                pg0 = psg.tile([128, 512], f32, tag="g")
                gru_layer([
                    wd_pairs(slice(0, 512)) + h_pairs0(slice(0, 512), range(5)),
                    wd_pairs(slice(512, 1024)) + h_pairs0(slice(512, 1024), range(5)),
                    wd_pairs(slice(1024, 1536)) + h_pairs0(slice(1024, 1536), [4]),
                    h_pairs0(slice(1536, 2048), range(5)),
                ], pg0)
                gru_math(pg0, h0m)
                transpose_to(h0m, [xh1T[:, 0:4, :], xh0T[:, 0:4, :]])

                # --- GRU layer 1 ---
                h_pairs1 = lambda cols, ks: [(xh1T[:, k, :], a1[:, k, cols])
                                             for k in ks]
                pg1 = psg.tile([128, 512], f32, tag="g")
                gru_layer([
                    h_pairs1(slice(0, 512), range(9)),
                    h_pairs1(slice(512, 1024), range(9)),
                    h_pairs1(slice(1024, 1536), [0, 1, 2, 3, 8]),
                    h_pairs1(slice(1536, 2048), [4, 5, 6, 7, 8]),
                ], pg1)
                gru_math(pg1, h1m)
                transpose_to(h1m, [xh1T[:, 4:8, :]])

                # --- out projection: [h1' | ws | cur+ones] @ Wo ---
                po = pss.tile([BS, T], f32, tag="s")
                opairs = [(xh1T[:, 4 + j, :], wo[:, j, :]) for j in range(4)] + \
                         [(wd[:, b, :], encW[:, b, 1536:1540]) for b in range(BS)] + \
                         [(xh0T[:, 4, :], wo[:, 4, :])]
                for i, (lhsT, rhs) in enumerate(opairs):
                    nc.tensor.matmul(po[:], lhsT, rhs, start=(i == 0),
                                     stop=(i == len(opairs) - 1))
                nc.vector.tensor_copy(outb[:, t, :], po[:])

                # --- cur update for next step ---
                if t < DEC - 1:
                    o8 = wk.tile([BS, T], f32, tag="o8")
                    nc.vector.tensor_copy(o8[:], po[:])
                    poT = pss.tile([T, BS], f32, tag="s")
                    nc.tensor.transpose(poT[:], o8[:], ident[:])
                    nc.vector.tensor_copy(xh0T[0:32, 4, :], inT[:, t, :])
                    nc.vector.tensor_copy(xh0T[0:T, 4, :], poT[:])

            nc.sync.dma_start(out_e[:], outb[:])

    # --- post-pass: walrus rejects instructions with more than a couple of
    # sync waits ("Too many sync wait commands").  Cap every instruction at
    # one wait by hoisting extras onto same-engine NoOps inserted just
    # before it (engine queues are in-order, so waiting earlier is safe). ---
    ctr = 0
    f = nc.m.functions[0]
    for blk in f.blocks:
        il = blk.instructions
        i = 0
        while i < len(il):
            inst = il[i]
            si = inst.sync_info
            waits = list(si.on_wait) if si is not None and si.on_wait else []
            if len(waits) > 1:
                SyncInfo = type(si)
                inst.sync_info = SyncInfo(
                    on_wait=[waits[-1]], on_update=list(si.on_update or []))
                for w in waits[:-1]:
                    nop = mybir.InstNoOp(name=f"I-nopw-{ctr}")
                    ctr += 1
                    nop.engine = inst.engine
                    nop.sync_info = SyncInfo(on_wait=[w], on_update=[])
                    nc.register_instruction(nop)
                    il.insert(i, nop)
                    i += 1
            i += 1

    return nc


def _prep_inputs(inputs, hidden, enc_outputs, target_indices,
                 W_attn, b_attn, v_attn,
                 gru_Wi0, gru_Wh0, gru_bi0, gru_bh0,
                 gru_Wi1, gru_Wh1, gru_bi1, gru_bh1,
                 W_out, b_out):
    """Build per-core input maps (host-side layout prep only)."""
    ti = np.asarray(target_indices)
    assert np.array_equal(ti, np.arange(T)), \
        "kernel specialized for target_indices == arange(T)"

    Wa_h = np.asarray(W_attn)[:, :H]
    Wa_e = np.asarray(W_attn)[:, H:]
    gru_Wi0 = np.asarray(gru_Wi0); gru_Wh0 = np.asarray(gru_Wh0)
    gru_bi0 = np.asarray(gru_bi0); gru_bh0 = np.asarray(gru_bh0)
    gru_Wi1 = np.asarray(gru_Wi1); gru_Wh1 = np.asarray(gru_Wh1)
    gru_bi1 = np.asarray(gru_bi1); gru_bh1 = np.asarray(gru_bh1)
    W_out = np.asarray(W_out); b_out = np.asarray(b_out)

    waeT = _bf16(Wa_e.T.reshape(4, 128, 512).transpose(1, 0, 2))
    wahT = _bf16(Wa_h.T.reshape(4, 128, 512).transpose(1, 0, 2))
    # block-diagonal v: vd[r, k, p, m] = v[k*128+r] iff m in {2p, 2p+1}
    v_rk = np.asarray(v_attn, np.float32).reshape(4, 128).T   # [128, 4]
    v_h = np.zeros((128, 4, 4, BS), np.float32)
    for p in range(4):
        v_h[:, :, p, 2 * p] = v_rk
        v_h[:, :, p, 2 * p + 1] = v_rk
    v_h = _bf16(v_h)

    # layer0 weights over z-order [h0(512); cur(32); one_a; one_b] (5 chunks)
    A0 = np.zeros((2048, 640), np.float32)
    A0[0:1024, 0:H] = gru_Wh0[0:1024]
    A0[0:1024, H:H + F] = gru_Wi0[0:1024, :F]
    A0[0:1024, 544] = gru_bi0[0:1024] + gru_bh0[0:1024]
    A0[1024:1536, H:H + F] = gru_Wi0[1024:1536, :F]
    A0[1024:1536, 544] = gru_bi0[1024:1536]
    A0[1536:2048, 0:H] = gru_Wh0[1024:1536]
    A0[1536:2048, 545] = gru_bh0[1024:1536]
    a0 = _bf16(A0.T.reshape(5, 128, 2048).transpose(1, 0, 2))

    # ws-part weights for the encW precompute: [r|z|i_n|out] rows x h-cols
    AW = np.zeros((1540, 512), np.float32)
    AW[0:1536, :] = gru_Wi0[:, F:]
    AW[1536:1540, :] = W_out[:, H:2 * H]
    aw = _bf16(AW.T.reshape(4, 128, 1540).transpose(1, 0, 2))

    # layer1 over z-order [h0'(512); h1(512); cur-slot unused; one_a; one_b]
    A1 = np.zeros((2048, 1152), np.float32)
    A1[0:1024, 0:H] = gru_Wi1[0:1024]
    A1[0:1024, H:2 * H] = gru_Wh1[0:1024]
    A1[0:1024, 1056] = gru_bi1[0:1024] + gru_bh1[0:1024]
    A1[1024:1536, 0:H] = gru_Wi1[1024:1536]
    A1[1024:1536, 1056] = gru_bi1[1024:1536]
    A1[1536:2048, H:2 * H] = gru_Wh1[1024:1536]
    A1[1536:2048, 1057] = gru_bh1[1024:1536]
    a1 = _bf16(A1.T.reshape(9, 128, 2048).transpose(1, 0, 2))

    # out projection: z-order [h1'(512) | (ws via encW) | cur(32); b_out]
    WoF = np.zeros((T, 640), np.float32)
    WoF[:, 0:H] = W_out[:, 0:H]
    WoF[:, H:H + F] = W_out[:, 2 * H:2 * H + F]
    WoF[:, 544] = b_out
    wo = _bf16(WoF.T.reshape(5, 128, T).transpose(1, 0, 2))

    battn = _bf16(np.asarray(b_attn)[None, :])
    onesr = _bf16(np.ones((1, BS * E), np.float32))
    ident = _f32(np.eye(8, dtype=np.float32))
    x1t8 = np.zeros((128, BS), np.float32)
    x1t8[32, :] = 1.0
    x1t8[33, :] = 1.0

    inputs = np.asarray(inputs)
    hidden = np.asarray(hidden)
    enc_outputs = np.asarray(enc_outputs)

    in_maps = []
    for c in range(N_CORES):
        s = slice(c * BS, (c + 1) * BS)
        encc = enc_outputs[s]                      # [8, 96, 512]
        encT = _bf16(encc.reshape(BS * E, H).T.reshape(4, 128, BS * E)
                     .transpose(1, 0, 2))
        h0 = hidden[0, s]                          # [8, 512]
        h1 = hidden[1, s]
        x0t8 = x1t8.copy()
        x0t8[0:F, :] = inputs[s, 0, :].T
        in_maps.append({
            "inputsT": _bf16(inputs[s].transpose(2, 1, 0)),
            "h0T": _bf16(h0.T.reshape(4, 128, BS).transpose(1, 0, 2)),
            "h1T": _bf16(h1.T.reshape(4, 128, BS).transpose(1, 0, 2)),
            "h0m": _f32(h0), "h1m": _f32(h1),
            "encT": encT,
            "waeT": waeT, "battn": battn, "onesr": onesr,
            "wahT": wahT, "v": v_h,
            "a0": a0, "aw": aw, "a1": a1, "wo": wo,
            "ident": ident,
            "x0t8": _bf16(x0t8), "x1t8": _bf16(x1t8),
        })
    return in_maps


def get_nc():
    if "nc" not in _COMPILED:
        _COMPILED["nc"] = build_nc()
    return _COMPILED["nc"]


def kernel(**inputs):
    from concourse.bass_utils import run_bass_kernel_spmd
    nc = get_nc()
    in_maps = _prep_inputs(**inputs)
    res = run_bass_kernel_spmd(nc, in_maps, list(range(N_CORES)))
    out = np.concatenate([res.results[c]["out"] for c in range(N_CORES)], axis=0)
    return np.ascontiguousarray(out, dtype=np.float32)



# revision 25
# speedup vs baseline: 3.8301x; 3.8301x over previous
"""Trainium2 Bass kernel for nn_DecoderWithAttention.

2-layer GRU decoder with Bahdanau attention, 12 sequential timesteps.
Strategy: data-parallel over batch (64 -> 8 cores x 8), weights replicated.

v2 design (cost-model-driven):
- All gate/out matmuls are "flipped": weights are the stationary operand
  [K=128 input-chunk, M=128 unit-chunk], the per-core batch state streams as
  the moving operand [128, 8].  Outputs land directly in transposed
  [unit, batch] layout, so GRU elementwise math runs on [128, 32]-free tiles
  and the hidden state never needs transposing.
- Sigmoid is computed as (1+tanh(x/2))/2 with the 1/2 folded into the
  host-prepped weights, so every activation is Tanh/Exp/Copy - one
  activation table, zero table reloads.
- Softmax runs in transposed [E, B] layout: per-batch score columns from
  tiny matmuls, exp on Act, partition_all_reduce + divide on GPSIMD.
- Attention context ws is materialized per batch with enc[b] stationary and
  the softmax column as a 1-wide moving operand.
- Setup DMAs are spread over 4 engine queues (sync/vector/scalar/gpsimd).
"""
import sys
sys.path.insert(0, '/opt/trn_rl_repo')
import numpy as np

B, DEC, F = 64, 12, 32
L, H = 2, 512
E, T = 96, 4
N_CORES = 8
BS = B // N_CORES  # 8 batches per core

_COMPILED = {}


def _f32(x):
    return np.ascontiguousarray(x, dtype=np.float32)


def _bf16(x):
    import ml_dtypes
    return np.ascontiguousarray(np.asarray(x, dtype=np.float32).astype(ml_dtypes.bfloat16))


def build_nc():
    import concourse.bass as bass
    import concourse.tile as tile
    from concourse import mybir, library_config
    from concourse.bass import bass_isa
    from concourse.vector_clock import ScopedClock

    f32 = mybir.dt.float32
    bf16 = mybir.dt.bfloat16
    AF = mybir.ActivationFunctionType
    ALU = mybir.AluOpType

    # --- patch: the TileContext exit drain gets >1 sem wait, which this
    # walrus rejects ("Too many sync wait commands"); split into
    # single-wait drains. ---
    def patched_drain(self, tick_clock, wait_clock):
        nc = self.nc
        drain_inst = nc.sync.drain()
        wait_clock.add_sem_waits(
            drain_inst.ins, ScopedClock({None: tick_clock.global_clock}))
        si = drain_inst.ins.sync_info
        waits = list(si.on_wait or [])
        if len(waits) > 1:
            SyncInfo = type(si)
            drain_inst.ins.sync_info = SyncInfo(
                on_wait=[waits[0]], on_update=list(si.on_update or []))
            for w in waits[1:]:
                d2 = nc.sync.drain()
                d2.ins.sync_info = SyncInfo(on_wait=[w], on_update=[])
        nc.all_engine_barrier()
        assert self.sems is not None
        popped = nc._tile_sem_poison_stack.pop()
        assert popped is self._sem_poison
        nc.clear_and_free_semaphores(list(self.sems.allocated().values()))
        nc.all_engine_barrier()

    tile.TileContext._drain_and_barrier = patched_drain

    nc = bass.Bass()

    def P(name, shape, dt=bf16):
        return nc.declare_dram_parameter(name, list(shape), dt, isOutput=False)

    # per-core inputs
    s0init_e = P("s0init", [128, 9, BS])
    s1init_e = P("s1init", [128, 5, BS])
    inT_e = P("inT", [F, DEC, BS])
    encT_e = P("encT", [128, 4, BS * E])        # [h'-chunk part, k, (b e)]
    encB_e = P("encB", [E, BS, 4, 128])         # [e, b, c, u]
    # replicated weights
    waeT_e = P("waeT", [128, 4, H])
    wahT_e = P("wahT", [128, 4, H])
    baT_e = P("baT", [1, H])
    ones8_e = P("ones8", [1, BS])
    vT_e = P("vT", [128, 4])
    onesc_e = P("onesc", [E, 1])
    onesr_e = P("onesr", [1, E])
    wL0_e = P("wL0", [128, 108, 128])   # R/Z/U x c x j(0..8)
    wL1_e = P("wL1", [128, 96, 128])    # R/Z/U x c x j(0..7)
    vb0_e = P("vb0", [1, 4, 128])       # L0 V-gate bias row (0.5*bh_n)
    b81_e = P("b81", [1, 12, 128])      # L1 R/Z/U bias rows
    vb1_e = P("vb1", [1, 4, 128])       # L1 V-gate bias row
    woT_e = P("woT", [128, 9, T])
    ident4_e = P("ident4", [T, T], f32)
    out_e = nc.declare_dram_parameter("out", [DEC, BS, T], f32, isOutput=True)

    with tile.TileContext(nc) as tc:
        with tc.tile_pool(name="wts", bufs=1) as wts, \
             tc.tile_pool(name="work", bufs=2) as wk, \
             tc.tile_pool(name="psG", bufs=1, space="PSUM") as psG, \
             tc.tile_pool(name="pss", bufs=4, space="PSUM") as pss:

            def load(pool, ext, shape, dt, q):
                t = pool.tile(list(shape), dt, tag=ext.name)
                q.dma_start(t[:], ext[:])
                return t

            # ---- DMAs on the 3 available queues (SP / Act / Pool) ----
            # sync: encP deps first, then a wL1 half
            encT = load(wts, encT_e, [128, 4, BS * E], bf16, nc.sync)
            waeT = load(wts, waeT_e, [128, 4, H], bf16, nc.sync)
            vT = load(wts, vT_e, [128, 4], bf16, nc.sync)
            baT = load(wts, baT_e, [1, H], bf16, nc.sync)
            ones8 = load(wts, ones8_e, [1, BS], bf16, nc.sync)
            ident4 = load(wts, ident4_e, [T, T], f32, nc.sync)
            onesc = load(wts, onesc_e, [E, 1], bf16, nc.sync)
            onesr = load(wts, onesr_e, [1, E], bf16, nc.sync)
            wL1 = wts.tile([128, 96, 128], bf16, tag="wL1")
            nc.sync.dma_start(wL1[:, 0:48, :], wL1_e[:, 0:48, :])
            # gpsimd: q deps, state, encB, wL1 second half
            wahT = load(wts, wahT_e, [128, 4, H], bf16, nc.gpsimd)
            S0 = load(wts, s0init_e, [128, 9, BS], bf16, nc.gpsimd)
            S1 = load(wts, s1init_e, [128, 5, BS], bf16, nc.gpsimd)
            inT = load(wts, inT_e, [F, DEC, BS], bf16, nc.gpsimd)
            woT = load(wts, woT_e, [128, 9, T], bf16, nc.gpsimd)
            vb0 = load(wts, vb0_e, [1, 4, 128], bf16, nc.gpsimd)
            b81 = load(wts, b81_e, [1, 12, 128], bf16, nc.gpsimd)
            vb1 = load(wts, vb1_e, [1, 4, 128], bf16, nc.gpsimd)
            encB = load(wts, encB_e, [E, BS, 4, 128], bf16, nc.gpsimd)
            nc.gpsimd.dma_start(wL1[:, 48:96, :], wL1_e[:, 48:96, :])
            # scalar: wL0 (Act is otherwise idle until the first tanh)
            wL0 = load(wts, wL0_e, [128, 108, 128], bf16, nc.scalar)

            # persistent tiles
            encP = wts.tile([128, 4, BS * E], bf16)   # enc @ Wa_e.T, T-layout
            outTb = wts.tile([T, DEC, BS], f32)

            # ---- one-time: encP[h,(b,e)] = Wa_e @ encT (no bias; folded
            # into q) ----
            for m in range(4):
                for n2 in range(2):
                    ns = slice(n2 * 384, (n2 + 1) * 384)
                    pe = pss.tile([128, 384], f32, tag="s")
                    for k in range(4):
                        nc.tensor.matmul(
                            pe[:], waeT[:, k, m * 128:(m + 1) * 128],
                            encT[:, k, ns], start=(k == 0), stop=(k == 3))
                    nc.vector.tensor_copy(encP[:, m, ns], pe[:])

            # moving-operand map for L1 / out-proj z-chunks
            def mv1(j):
                if j < 4:
                    return S0[:, j, :]       # h0'
                if j < 8:
                    return S1[:, j - 4, :]   # h1
                return S1[:, 4, :]           # ones row

            # L0 pre (no attention dep): ph0 = [U-pre | V] over h0/bias
            # chunks; V reuses U's 0.5*Wh_n blocks.  R/Z run fully post-
            # attention (all 9 chunks) so tanh can read their psum directly.
            def gates_pre0(ph):
                for c in range(4):
                    for ji, j in enumerate([0, 1, 2, 3, 8]):
                        nc.tensor.matmul(
                            ph[:, c, :], wL0[:, 72 + c * 9 + j, :],
                            S0[:, j, :], start=(ji == 0), stop=(ji == 4))
                    for ji, j in enumerate([0, 1, 2, 3]):
                        nc.tensor.matmul(
                            ph[:, 4 + c, :], wL0[:, 72 + c * 9 + j, :],
                            S0[:, j, :], start=(ji == 0), stop=False)
                    nc.tensor.matmul(
                        ph[:, 4 + c, :], vb0[0:1, c, :],
                        ones8[0:1, :], start=False, stop=True)

            def gates_post0(pw):
                for c in range(4):
                    for g, base in ((0, 0), (1, 36)):
                        for ji, j in enumerate(range(9)):
                            nc.tensor.matmul(
                                pw[:, g * 4 + c, :],
                                wL0[:, base + c * 9 + j, :],
                                S0[:, j, :], start=(ji == 0), stop=(ji == 8))
                    for ji, j in enumerate([4, 5, 6, 7]):
                        nc.tensor.matmul(
                            pw[:, 8 + c, :], wL0[:, 72 + c * 9 + j, :],
                            S0[:, j, :], start=(ji == 0), stop=(ji == 3))

            # L1: pre = [U-pre over h1/bias | V]; post = [R | Z | U-h0'].
            def gates_pre1(ph):
                for c in range(4):
                    for ji, j in enumerate([4, 5, 6, 7]):
                        nc.tensor.matmul(
                            ph[:, c, :], wL1[:, 64 + c * 8 + j, :],
                            mv1(j), start=(ji == 0), stop=False)
                    nc.tensor.matmul(
                        ph[:, c, :], b81[0:1, 8 + c, :],
                        S1[0:1, 4, :], start=False, stop=True)
                    for ji, j in enumerate([4, 5, 6, 7]):
                        nc.tensor.matmul(
                            ph[:, 4 + c, :], wL1[:, 64 + c * 8 + j, :],
                            mv1(j), start=(ji == 0), stop=False)
                    nc.tensor.matmul(
                        ph[:, 4 + c, :], vb1[0:1, c, :],
                        S1[0:1, 4, :], start=False, stop=True)

            def gates_post1(pw):
                for c in range(4):
                    for g, base in ((0, 0), (1, 32)):
                        for ji, j in enumerate(range(8)):
                            nc.tensor.matmul(
                                pw[:, g * 4 + c, :],
                                wL1[:, base + c * 8 + j, :],
                                mv1(j), start=(ji == 0), stop=False)
                        nc.tensor.matmul(
                            pw[:, g * 4 + c, :], b81[0:1, g * 4 + c, :],
                            S1[0:1, 4, :], start=False, stop=True)
                    for ji, j in enumerate([0, 1, 2, 3]):
                        nc.tensor.matmul(
                            pw[:, 8 + c, :], wL1[:, 64 + c * 8 + j, :],
                            mv1(j), start=(ji == 0), stop=(ji == 3))

            def gru_math(ph, pw, S, tag):
                """ph: [128,8,8] psum = [U-pre | V]; pw: [128,12,8] psum =
                [R | Z | U-ws].  r=(1+tanh(R/2))/2 etc.; U = i_n + 0.5*h_n;
                V = 0.5*h_n.  n = tanh(U + r'*V); h' = n + 0.5*(1+z')*(h-n).
                """
                rz = wk.tile([128, 8, BS], bf16, tag="rz" + tag)
                nc.scalar.activation(rz[:], pw[:, 0:8, :], AF.Tanh, scale=0.5)
                mm = wk.tile([128, 4, BS], bf16, tag="mm" + tag)
                nc.vector.tensor_mul(mm[:], rz[:, 0:4, :], ph[:, 4:8, :])
                tt = wk.tile([128, 4, BS], f32, tag="tt" + tag)
                nc.vector.tensor_add(tt[:], mm[:], ph[:, 0:4, :])
                t2 = wk.tile([128, 4, BS], f32, tag="t2" + tag)
                nc.vector.tensor_add(t2[:], tt[:], pw[:, 8:12, :])
                nn = wk.tile([128, 4, BS], bf16, tag="nn" + tag)
                nc.scalar.activation(nn[:], t2[:], AF.Tanh)
                dd = wk.tile([128, 4, BS], bf16, tag="dd" + tag)
                nc.gpsimd.tensor_sub(dd[:], S[:, 0:4, :], nn[:])
                ee = wk.tile([128, 4, BS], bf16, tag="ee" + tag)
                nc.vector.scalar_tensor_tensor(
                    ee[:], rz[:, 4:8, :], 1.0, dd[:], ALU.add, ALU.mult)
                nc.vector.scalar_tensor_tensor(
                    S[:, 0:4, :], ee[:], 0.5, nn[:], ALU.mult, ALU.add)

            # ================= time loop =================
            for t in range(DEC):
                # --- q = Wa_h @ h1 + b_attn : psum [128,4,8] ---
                pq = pss.tile([128, 4, BS], f32, tag="s")
                for m in range(4):
                    for k in range(4):
                        nc.tensor.matmul(
                            pq[:, m, :], wahT[:, k, m * 128:(m + 1) * 128],
                            S1[:, k, :], start=(k == 0), stop=False)
                    nc.tensor.matmul(
                        pq[:, m, :], baT[0:1, m * 128:(m + 1) * 128],
                        ones8[0:1, :], start=False, stop=True)
                qb = wk.tile([128, 4, BS], bf16, tag="qb")
                nc.vector.tensor_copy(qb[:], pq[:])

                # --- energy = tanh(encP + q bcast), 2 halves over h-chunks ---
                ea = wk.tile([128, 4, BS * E], bf16, tag="ea")
                en = wk.tile([128, 4, BS * E], bf16, tag="en")
                for hh in range(2):
                    for mi, eng in ((2 * hh, nc.vector), (2 * hh + 1, nc.gpsimd)):
                        ms = slice(mi, mi + 1)
                        eng.tensor_add(
                            ea[:, ms, :].rearrange("p m (b e) -> p m b e", b=BS),
                            encP[:, ms, :].rearrange("p m (b e) -> p m b e", b=BS),
                            qb[:, ms, :].unsqueeze(3).broadcast_to(
                                (128, 1, BS, E)))
                    nc.scalar.activation(en[:, 2 * hh:2 * hh + 2, :],
                                         ea[:, 2 * hh:2 * hh + 2, :], AF.Tanh)

                # --- pre-gates (no attention dep): L0 j in {0..3,8},
                #     L1 j in {4..8}; V gates complete here ---
                ph0 = psG.tile([128, 8, BS], f32, tag="ph0")
                ph1 = psG.tile([128, 8, BS], f32, tag="ph1")
                pw0 = psG.tile([128, 12, BS], f32, tag="pw0")
                pw1 = psG.tile([128, 12, BS], f32, tag="pw1")
                gates_pre0(ph0)
                gates_pre1(ph1)

                # --- scoresT [E, 8]: per-batch column, 4 consecutive
                # k-accumulating matmuls (one open psum group at a time) ---
                scT = pss.tile([E, BS], f32, tag="s")
                for b in range(BS):
                    for k in range(4):
                        nc.tensor.matmul(
                            scT[:, b:b + 1], en[:, k, b * E:(b + 1) * E],
                            vT[:, k:k + 1], start=(k == 0), stop=(k == 3))

                # --- softmax over E (partition dim); scores small: no
                # max-subtraction.  Denominator via ones-column matmul,
                # partition-broadcast of 1/Z via K=1 ones-row matmul. ---
                ex = wk.tile([E, BS], bf16, tag="ex")
                nc.scalar.activation(ex[:], scT[:], AF.Exp)
                Zp = pss.tile([1, BS], f32, tag="s")
                nc.tensor.matmul(Zp[:], onesc[:], ex[:], start=True, stop=True)
                rr = wk.tile([1, BS], bf16, tag="rr")
                with nc.allow_low_precision(reason="bf16 softmax weights"):
                    nc.vector.reciprocal(rr[:], Zp[:])
                rbc = pss.tile([E, BS], f32, tag="s")
                nc.tensor.matmul(rbc[:], onesr[:], rr[:], start=True, stop=True)
                wN = wk.tile([E, BS], bf16, tag="wN")
                nc.vector.tensor_mul(wN[:], ex[:], rbc[:])

                # --- wsT [128,4,8]: enc[b] stationary x softmax column ---
                wsP = pss.tile([128, 4, BS], f32, tag="s")
                for c in range(4):
                    for b in range(BS):
                        nc.tensor.matmul(
                            wsP[:, c, b:b + 1], encB[:, b, c, :],
                            wN[:, b:b + 1], start=True, stop=True)
                nc.vector.tensor_copy(S0[:, 4:8, :], wsP[:])

                # --- L0: ws-dependent gate parts, then math ---
                gates_post0(pw0)
                gru_math(ph0, pw0, S0, "0")

                # --- L1: h0'-dependent gate parts, then math ---
                gates_post1(pw1)
                gru_math(ph1, pw1, S1, "1")

                # --- out projection (transposed): [T, 8] ---
                po = pss.tile([T, BS], f32, tag="s")
                for j in range(9):
                    mvo = S1[:, j, :] if j < 4 else S0[:, j, :]
                    nc.tensor.matmul(po[:], woT[:, j, :], mvo,
                                     start=(j == 0), stop=(j == 8))
                nc.vector.tensor_copy(outTb[:, t, :], po[:])

                # --- cur update for next step ---
                if t < DEC - 1:
                    nc.gpsimd.tensor_copy(S0[0:F, 8, :], inT[:, t, :])
                    nc.vector.tensor_copy(S0[0:T, 8, :], po[:])

            # --- final: transpose [T,(t b)] -> [(t b),T], DMA out ---
            pfin = pss.tile([DEC * BS, T], f32, tag="s")
            nc.tensor.transpose(
                pfin[:], outTb[:].rearrange("T t b -> T (t b)"), ident4[:])
            osb = wk.tile([DEC * BS, T], f32, tag="osb")
            nc.vector.tensor_copy(osb[:], pfin[:])
            nc.sync.dma_start(out_e[:].rearrange("t b T -> (t b) T"), osb[:])

    # --- post-pass: walrus rejects instructions with more than a couple of
    # sync waits ("Too many sync wait commands").  Cap every instruction at
    # one wait by hoisting extras onto same-engine NoOps inserted just
    # before it (engine queues are in-order, so waiting earlier is safe). ---
    from concourse import mybir
    ctr = 0
    f = nc.m.functions[0]
    for blk in f.blocks:
        il = blk.instructions
        i = 0
        while i < len(il):
            inst = il[i]
            si = inst.sync_info
            waits = list(si.on_wait) if si is not None and si.on_wait else []
            if len(waits) > 1:
                SyncInfo = type(si)
                inst.sync_info = SyncInfo(
                    on_wait=[waits[-1]], on_update=list(si.on_update or []))
                for w in waits[:-1]:
                    nop = mybir.InstNoOp(name=f"I-nopw-{ctr}")
                    ctr += 1
                    nop.engine = inst.engine
                    nop.sync_info = SyncInfo(on_wait=[w], on_update=[])
                    nc.register_instruction(nop)
                    il.insert(i, nop)
                    i += 1
            i += 1

    return nc


def _prep_inputs(inputs, hidden, enc_outputs, target_indices,
                 W_attn, b_attn, v_attn,
                 gru_Wi0, gru_Wh0, gru_bi0, gru_bh0,
                 gru_Wi1, gru_Wh1, gru_bi1, gru_bh1,
                 W_out, b_out):
    """Build per-core input maps (host-side layout prep only)."""
    ti = np.asarray(target_indices)
    assert np.array_equal(ti, np.arange(T)), \
        "kernel specialized for target_indices == arange(T)"

    Wa_h = np.asarray(W_attn, np.float32)[:, :H]
    Wa_e = np.asarray(W_attn, np.float32)[:, H:]
    b_attn = np.asarray(b_attn, np.float32)
    v_attn = np.asarray(v_attn, np.float32)
    Wi0 = np.asarray(gru_Wi0, np.float32); Wh0 = np.asarray(gru_Wh0, np.float32)
    bi0 = np.asarray(gru_bi0, np.float32); bh0 = np.asarray(gru_bh0, np.float32)
    Wi1 = np.asarray(gru_Wi1, np.float32); Wh1 = np.asarray(gru_Wh1, np.float32)
    bi1 = np.asarray(gru_bi1, np.float32); bh1 = np.asarray(gru_bh1, np.float32)
    W_out = np.asarray(W_out, np.float32); b_out = np.asarray(b_out, np.float32)

    waeT = _bf16(Wa_e.T.reshape(4, 128, H).transpose(1, 0, 2))
    wahT = _bf16(Wa_h.T.reshape(4, 128, H).transpose(1, 0, 2))
    baT = _bf16(b_attn[None, :])
    ones8 = _bf16(np.ones((1, BS), np.float32))
    vT = _bf16(v_attn.reshape(4, 128).T)
    ident4 = _f32(np.eye(T, dtype=np.float32))
    onesc = _bf16(np.ones((E, 1), np.float32))
    onesr = _bf16(np.ones((1, E), np.float32))

    # --- L0 gate weight blocks: z-order [h0(512) | ws(512) | cur(32),
    # one(@1056)]; stationary block (j,c) = Z[128j:128j+128, 128c:128c+128]
    Z0R = np.zeros((1152, H), np.float32)
    Z0R[0:512] = Wh0[0:512].T
    Z0R[512:1024] = Wi0[0:512, F:].T
    Z0R[1024:1056] = Wi0[0:512, 0:F].T
    Z0R[1056] = bi0[0:512] + bh0[0:512]
    Z0Z = np.zeros((1152, H), np.float32)
    Z0Z[0:512] = Wh0[512:1024].T
    Z0Z[512:1024] = Wi0[512:1024, F:].T
    Z0Z[1024:1056] = Wi0[512:1024, 0:F].T
    Z0Z[1056] = bi0[512:1024] + bh0[512:1024]
    Z0U = np.zeros((1152, H), np.float32)
    Z0U[0:512] = 0.5 * Wh0[1024:1536].T
    Z0U[512:1024] = Wi0[1024:1536, F:].T
    Z0U[1024:1056] = Wi0[1024:1536, 0:F].T
    Z0U[1056] = bi0[1024:1536] + 0.5 * bh0[1024:1536]
    Z0V = np.zeros((1152, H), np.float32)
    Z0V[0:512] = 0.5 * Wh0[1024:1536].T
    Z0V[1056] = 0.5 * bh0[1024:1536]

    # --- L1: z-order [h0'(512) | h1(512) | one(@1024)] ---
    Z1R = np.zeros((1152, H), np.float32)
    Z1R[0:512] = Wi1[0:512].T
    Z1R[512:1024] = Wh1[0:512].T
    Z1R[1024] = bi1[0:512] + bh1[0:512]
    Z1Z = np.zeros((1152, H), np.float32)
    Z1Z[0:512] = Wi1[512:1024].T
    Z1Z[512:1024] = Wh1[512:1024].T
    Z1Z[1024] = bi1[512:1024] + bh1[512:1024]
    Z1U = np.zeros((1152, H), np.float32)
    Z1U[0:512] = Wi1[1024:1536].T
    Z1U[512:1024] = 0.5 * Wh1[1024:1536].T
    Z1U[1024] = bi1[1024:1536] + 0.5 * bh1[1024:1536]
    Z1V = np.zeros((1152, H), np.float32)
    Z1V[512:1024] = 0.5 * Wh1[1024:1536].T
    Z1V[1024] = 0.5 * bh1[1024:1536]

    def blocks(mats, js):
        blks = []
        for Zm in mats:
            for c in range(4):
                for j in js:
                    blks.append(Zm[128 * j:128 * j + 128,
                                   128 * c:128 * c + 128])
        return _bf16(np.stack(blks).transpose(1, 0, 2))

    wL0 = blocks((Z0R, Z0Z, Z0U), range(9))
    wL1 = blocks((Z1R, Z1Z, Z1U), range(8))
    vb0 = _bf16(Z0V[1056].reshape(1, 4, 128))
    vb1 = _bf16(Z1V[1024].reshape(1, 4, 128))
    b81 = _bf16(np.stack([Zm[1024, 128 * c:128 * c + 128]
                          for Zm in (Z1R, Z1Z, Z1U) for c in range(4)])[None])

    # --- out projection: z-order [h1'(512) | ws(512) | cur(32), one(@1056)]
    ZO = np.zeros((1152, T), np.float32)
    ZO[0:512] = W_out[:, 0:H].T
    ZO[512:1024] = W_out[:, H:2 * H].T
    ZO[1024:1056] = W_out[:, 2 * H:2 * H + F].T
    ZO[1056] = b_out
    woT = _bf16(ZO.reshape(9, 128, T).transpose(1, 0, 2))

    inputs = np.asarray(inputs, np.float32)
    hidden = np.asarray(hidden, np.float32)
    enc_outputs = np.asarray(enc_outputs, np.float32)

    in_maps = []
    for cc in range(N_CORES):
        s = slice(cc * BS, (cc + 1) * BS)
        encc = enc_outputs[s]                      # [8, 96, 512]
        encT = _bf16(encc.reshape(BS * E, H).T.reshape(4, 128, BS * E)
                     .transpose(1, 0, 2))
        encB = _bf16(encc.transpose(1, 0, 2).reshape(E, BS, 4, 128))
        h0 = hidden[0, s]                          # [8, 512]
        h1 = hidden[1, s]
        s0init = np.zeros((128, 9, BS), np.float32)
        s0init[:, 0:4, :] = h0.T.reshape(4, 128, BS).transpose(1, 0, 2)
        s0init[0:F, 8, :] = inputs[s, 0, :].T
        s0init[F, 8, :] = 1.0
        s1init = np.zeros((128, 5, BS), np.float32)
        s1init[:, 0:4, :] = h1.T.reshape(4, 128, BS).transpose(1, 0, 2)
        s1init[0, 4, :] = 1.0
        in_maps.append({
            "s0init": _bf16(s0init), "s1init": _bf16(s1init),
            "inT": _bf16(inputs[s].transpose(2, 1, 0)),
            "encT": encT, "encB": encB,
            "waeT": waeT, "wahT": wahT, "baT": baT, "ones8": ones8,
            "vT": vT, "wL0": wL0, "wL1": wL1, "woT": woT, "ident4": ident4,
            "onesc": onesc, "onesr": onesr,
            "vb0": vb0, "vb1": vb1, "b81": b81,
        })
    return in_maps


def get_nc():
    if "nc" not in _COMPILED:
        _COMPILED["nc"] = build_nc()
    return _COMPILED["nc"]


def kernel(**inputs):
    from concourse.bass_utils import run_bass_kernel_spmd
    nc = get_nc()
    in_maps = _prep_inputs(**inputs)
    res = run_bass_kernel_spmd(nc, in_maps, list(range(N_CORES)))
    out = np.concatenate([res.results[c]["out"].transpose(1, 0, 2)
                          for c in range(N_CORES)], axis=0)
    return np.ascontiguousarray(out, dtype=np.float32)


# revision 31
# speedup vs baseline: 3.9348x; 1.0273x over previous
"""Trainium2 Bass kernel for nn_DecoderWithAttention.

2-layer GRU decoder with Bahdanau attention, 12 sequential timesteps.
Strategy: data-parallel over batch (64 -> 8 cores x 8), weights replicated.

v2 design (cost-model-driven):
- All gate/out matmuls are "flipped": weights are the stationary operand
  [K=128 input-chunk, M=128 unit-chunk], the per-core batch state streams as
  the moving operand [128, 8].  Outputs land directly in transposed
  [unit, batch] layout, so GRU elementwise math runs on [128, 32]-free tiles
  and the hidden state never needs transposing.
- Sigmoid is computed as (1+tanh(x/2))/2 with the 1/2 folded into the
  host-prepped weights, so every activation is Tanh/Exp/Copy - one
  activation table, zero table reloads.
- Softmax runs in transposed [E, B] layout: per-batch score columns from
  tiny matmuls, exp on Act, partition_all_reduce + divide on GPSIMD.
- Attention context ws is materialized per batch with enc[b] stationary and
  the softmax column as a 1-wide moving operand.
- Setup DMAs are spread over 4 engine queues (sync/vector/scalar/gpsimd).
"""
import sys
sys.path.insert(0, '/opt/trn_rl_repo')
import numpy as np

B, DEC, F = 64, 12, 32
L, H = 2, 512
E, T = 96, 4
N_CORES = 8
BS = B // N_CORES  # 8 batches per core

_COMPILED = {}


def _f32(x):
    return np.ascontiguousarray(x, dtype=np.float32)


def _bf16(x):
    import ml_dtypes
    return np.ascontiguousarray(np.asarray(x, dtype=np.float32).astype(ml_dtypes.bfloat16))


def build_nc():
    import concourse.bass as bass
    import concourse.tile as tile
    from concourse import mybir, library_config
    from concourse.bass import bass_isa
    from concourse.vector_clock import ScopedClock

    f32 = mybir.dt.float32
    bf16 = mybir.dt.bfloat16
    AF = mybir.ActivationFunctionType
    ALU = mybir.AluOpType

    # --- patch: the TileContext exit drain gets >1 sem wait, which this
    # walrus rejects ("Too many sync wait commands"); split into
    # single-wait drains. ---
    def patched_drain(self, tick_clock, wait_clock):
        nc = self.nc
        drain_inst = nc.sync.drain()
        wait_clock.add_sem_waits(
            drain_inst.ins, ScopedClock({None: tick_clock.global_clock}))
        si = drain_inst.ins.sync_info
        waits = list(si.on_wait or [])
        if len(waits) > 1:
            SyncInfo = type(si)
            drain_inst.ins.sync_info = SyncInfo(
                on_wait=[waits[0]], on_update=list(si.on_update or []))
            for w in waits[1:]:
                d2 = nc.sync.drain()
                d2.ins.sync_info = SyncInfo(on_wait=[w], on_update=[])
        nc.all_engine_barrier()
        assert self.sems is not None
        popped = nc._tile_sem_poison_stack.pop()
        assert popped is self._sem_poison
        nc.clear_and_free_semaphores(list(self.sems.allocated().values()))
        nc.all_engine_barrier()

    tile.TileContext._drain_and_barrier = patched_drain

    nc = bass.Bass()

    def P(name, shape, dt=bf16):
        return nc.declare_dram_parameter(name, list(shape), dt, isOutput=False)

    # per-core inputs
    s0init_e = P("s0init", [128, 9, BS])
    s1init_e = P("s1init", [128, 5, BS])
    inT_e = P("inT", [F, DEC, BS])
    encT_e = P("encT", [128, 4, BS * E])        # [h'-chunk part, k, (b e)]
    encB_e = P("encB", [E, BS, 4, 128])         # [e, b, c, u]
    # replicated weights
    waeT_e = P("waeT", [128, 4, H])
    wahT_e = P("wahT", [128, 4, H])
    # misc: [:,0:4]=vT, [:,4:100]=ones, [0,104:616]=b_attn
    misc_e = P("misc", [128, 616])
    wL0_e = P("wL0", [128, 108, 128])   # R/Z/U x c x j(0..8)
    wL1_e = P("wL1", [128, 96, 128])    # R/Z/U x c x j(0..7)
    bias3_e = P("bias3", [1, 20, 128])  # [vb0(4) | b81(12) | vb1(4)]
    woT_e = P("woT", [128, 9, T])
    ident4_e = P("ident4", [T, T], f32)
    out_e = nc.declare_dram_parameter("out", [DEC, BS, T], f32, isOutput=True)

    with tile.TileContext(nc) as tc:
        with tc.tile_pool(name="wts", bufs=1) as wts, \
             tc.tile_pool(name="work", bufs=2) as wk, \
             tc.tile_pool(name="psG", bufs=1, space="PSUM") as psG, \
             tc.tile_pool(name="pss", bufs=4, space="PSUM") as pss:

            def load(pool, ext, shape, dt, q):
                t = pool.tile(list(shape), dt, tag=ext.name)
                q.dma_start(t[:], ext[:])
                return t

            # ---- DMAs on the 3 available queues (SP / Act / Pool),
            # ordered so encP inputs, wL0 and the U/V half of wL1 land
            # first; small params are packed into misc/bias3 to avoid the
            # 500ns-per-descriptor minimum. ----
            encT = load(wts, encT_e, [128, 4, BS * E], bf16, nc.sync)
            waeT = load(wts, waeT_e, [128, 4, H], bf16, nc.sync)
            wL1 = wts.tile([128, 96, 128], bf16, tag="wL1")
            nc.sync.dma_start(wL1[:, 64:96, :], wL1_e[:, 64:96, :])
            nc.sync.dma_start(wL1[:, 0:64, :], wL1_e[:, 0:64, :])

            wL0 = wts.tile([128, 108, 128], bf16, tag="wL0")
            nc.scalar.dma_start(wL0[:, 0:54, :], wL0_e[:, 0:54, :])
            misc = load(wts, misc_e, [128, 616], bf16, nc.scalar)
            ident4 = load(wts, ident4_e, [T, T], f32, nc.scalar)

            wahT = load(wts, wahT_e, [128, 4, H], bf16, nc.gpsimd)
            S0 = load(wts, s0init_e, [128, 9, BS], bf16, nc.gpsimd)
            S1 = load(wts, s1init_e, [128, 5, BS], bf16, nc.gpsimd)
            bias3 = load(wts, bias3_e, [1, 20, 128], bf16, nc.gpsimd)
            inT = load(wts, inT_e, [F, DEC, BS], bf16, nc.gpsimd)
            woT = load(wts, woT_e, [128, 9, T], bf16, nc.gpsimd)
            encB = load(wts, encB_e, [E, BS, 4, 128], bf16, nc.gpsimd)
            nc.gpsimd.dma_start(wL0[:, 54:108, :], wL0_e[:, 54:108, :])

            # persistent tiles
            encP = wts.tile([128, 4, BS * E], bf16)   # enc @ Wa_e.T, T-layout
            outTb = wts.tile([T, DEC, BS], f32)

            # pin the {Exp,Tanh,Copy} activation table during setup so no
            # per-step op pays the table load
            warm = wk.tile([1, BS], f32, tag="warm")
            nc.scalar.activation(warm[:], misc[0:1, 4:12], AF.Exp)
            nc.scalar.activation(warm[:], misc[0:1, 4:12], AF.Tanh)

            # ---- one-time: encP[h,(b,e)] = Wa_e @ encT (no bias; folded
            # into q) ----
            for m in range(4):
                for n2 in range(2):
                    ns = slice(n2 * 384, (n2 + 1) * 384)
                    pe = pss.tile([128, 384], f32, tag="s")
                    for k in range(4):
                        nc.tensor.matmul(
                            pe[:], waeT[:, k, m * 128:(m + 1) * 128],
                            encT[:, k, ns], start=(k == 0), stop=(k == 3))
                    if (m + n2) % 2 == 0:
                        nc.vector.tensor_copy(encP[:, m, ns], pe[:])
                    else:
                        nc.scalar.activation(encP[:, m, ns], pe[:], AF.Copy)

            # moving-operand map for L1 / out-proj z-chunks
            def mv1(j):
                if j < 4:
                    return S0[:, j, :]       # h0'
                if j < 8:
                    return S1[:, j - 4, :]   # h1
                return S1[:, 4, :]           # ones row

            # L0 pre (no attention dep): ph0 = V (0.5*Wh_n via U's
            # blocks + bias row).  R/Z/U accumulate fully post-attention,
            # with the h/bias chunks first so they prestream while the
            # softmax finishes (each group is consecutive start->stop).
            def gates_pre0(ph):
                for c in range(4):
                    for ji, j in enumerate([0, 1, 2, 3]):
                        nc.tensor.matmul(
                            ph[:, c, :], wL0[:, 72 + c * 9 + j, :],
                            S0[:, j, :], start=(ji == 0), stop=False)
                    nc.tensor.matmul(
                        ph[:, c, :], bias3[0:1, c, :],
                        misc[0:1, 4:12], start=False, stop=True)

            def gates_post0(pw):
                for g, base in ((0, 0), (1, 36), (2, 72)):
                    for c in range(4):
                        for ji, j in enumerate([0, 1, 2, 3, 8, 4, 5, 6, 7]):
                            nc.tensor.matmul(
                                pw[:, g * 4 + c, :],
                                wL0[:, base + c * 9 + j, :],
                                S0[:, j, :], start=(ji == 0), stop=(ji == 8))

            # L1: ph1 = V (h1 chunks + bias); post = R/Z/U with h1/bias
            # chunks first (prestream during L0 math), h0' chunks last.
            def gates_pre1(ph):
                for c in range(4):
                    for ji, j in enumerate([4, 5, 6, 7]):
                        nc.tensor.matmul(
                            ph[:, c, :], wL1[:, 64 + c * 8 + j, :],
                            mv1(j), start=(ji == 0), stop=False)
                    nc.tensor.matmul(
                        ph[:, c, :], bias3[0:1, 16 + c, :],
                        S1[0:1, 4, :], start=False, stop=True)

            def gates_post1(pw):
                for g, base in ((0, 0), (1, 32), (2, 64)):
                    for c in range(4):
                        for ji, j in enumerate([4, 5, 6, 7]):
                            nc.tensor.matmul(
                                pw[:, g * 4 + c, :],
                                wL1[:, base + c * 8 + j, :],
                                mv1(j), start=(ji == 0), stop=False)
                        nc.tensor.matmul(
                            pw[:, g * 4 + c, :], bias3[0:1, 4 + g * 4 + c, :],
                            S1[0:1, 4, :], start=False, stop=False)
                        for ji, j in enumerate([0, 1, 2, 3]):
                            nc.tensor.matmul(
                                pw[:, g * 4 + c, :],
                                wL1[:, base + c * 8 + j, :],
                                mv1(j), start=False, stop=(ji == 3))

            def gru_math(ph, pw, S, tag):
                """ph: [128,4,8] psum = V = 0.5*h_n; pw: [128,12,8] psum =
                [R | Z | U] with U = i_n + 0.5*h_n.
                r=(1+tanh(R/2))/2 etc.; n = tanh(U + r'*V);
                h' = n + 0.5*(1+z')*(h-n)."""
                rz = wk.tile([128, 8, BS], bf16, tag="rz" + tag)
                nc.scalar.activation(rz[:], pw[:, 0:8, :], AF.Tanh, scale=0.5)
                mm = wk.tile([128, 4, BS], bf16, tag="mm" + tag)
                nc.vector.tensor_mul(mm[:], rz[:, 0:4, :], ph[:, 0:4, :])
                tt = wk.tile([128, 4, BS], f32, tag="tt" + tag)
                nc.vector.tensor_add(tt[:], mm[:], pw[:, 8:12, :])
                nn = wk.tile([128, 4, BS], bf16, tag="nn" + tag)
                nc.scalar.activation(nn[:], tt[:], AF.Tanh)
                dd = wk.tile([128, 4, BS], bf16, tag="dd" + tag)
                nc.vector.tensor_sub(dd[:], S[:, 0:4, :], nn[:])
                ee = wk.tile([128, 4, BS], bf16, tag="ee" + tag)
                nc.vector.scalar_tensor_tensor(
                    ee[:], rz[:, 4:8, :], 1.0, dd[:], ALU.add, ALU.mult)
                nc.vector.scalar_tensor_tensor(
                    S[:, 0:4, :], ee[:], 0.5, nn[:], ALU.mult, ALU.add)

            # ================= time loop =================
            for t in range(DEC):
                # --- q = Wa_h @ h1 + b_attn : psum [128,4,8] ---
                pq = pss.tile([128, 4, BS], f32, tag="s")
                for m in range(4):
                    for k in range(4):
                        nc.tensor.matmul(
                            pq[:, m, :], wahT[:, k, m * 128:(m + 1) * 128],
                            S1[:, k, :], start=(k == 0), stop=False)
                    nc.tensor.matmul(
                        pq[:, m, :], misc[0:1, 104 + m * 128:104 + (m + 1) * 128],
                        misc[0:1, 4:12], start=False, stop=True)
                qb = wk.tile([128, 4, BS], bf16, tag="qb")
                nc.scalar.activation(qb[:], pq[:], AF.Copy)

                # --- energy = tanh(encP + q bcast), 2 halves over h-chunks ---
                ea = wk.tile([128, 4, BS * E], bf16, tag="ea")
                en = wk.tile([128, 4, BS * E], bf16, tag="en")
                for hh in range(2):
                    for mi, eng in ((2 * hh, nc.vector), (2 * hh + 1, nc.gpsimd)):
                        ms = slice(mi, mi + 1)
                        eng.tensor_add(
                            ea[:, ms, :].rearrange("p m (e b) -> p m e b", e=E),
                            encP[:, ms, :].rearrange("p m (e b) -> p m e b", e=E),
                            qb[:, ms, :].unsqueeze(2).broadcast_to(
                                (128, 1, E, BS)))
                    nc.scalar.activation(en[:, 2 * hh:2 * hh + 2, :],
                                         ea[:, 2 * hh:2 * hh + 2, :], AF.Tanh)

                # --- pre-gates (no attention dep): L0 j in {0..3,8},
                #     L1 j in {4..8}; V gates complete here ---
                ph0 = psG.tile([128, 4, BS], f32, tag="ph0")
                ph1 = psG.tile([128, 4, BS], f32, tag="ph1")
                pw0 = psG.tile([128, 12, BS], f32, tag="pw0")
                pw1 = psG.tile([128, 12, BS], f32, tag="pw1")
                gates_pre0(ph0)
                gates_pre1(ph1)

                # --- scoresT [E, 8]: per-batch column, 4 consecutive
                # k-accumulating matmuls (one open psum group at a time) ---
                scT = pss.tile([E, BS], f32, tag="s")
                for b in range(BS):
                    for k in range(4):
                        nc.tensor.matmul(
                            scT[:, b:b + 1], en[:, k, b * E:(b + 1) * E],
                            misc[:, k:k + 1], start=(k == 0), stop=(k == 3))

                # --- softmax over E (partition dim); scores small: no
                # max-subtraction.  Denominator via ones-column matmul,
                # partition-broadcast of 1/Z via K=1 ones-row matmul. ---
                ex = wk.tile([E, BS], bf16, tag="ex")
                nc.scalar.activation(ex[:], scT[:], AF.Exp)
                Zp = pss.tile([1, BS], f32, tag="s")
                nc.tensor.matmul(Zp[:], misc[0:E, 4:5], ex[:], start=True, stop=True)
                rr = wk.tile([1, BS], bf16, tag="rr")
                with nc.allow_low_precision(reason="bf16 softmax weights"):
                    nc.vector.reciprocal(rr[:], Zp[:])
                rbc = pss.tile([E, BS], f32, tag="s")
                nc.tensor.matmul(rbc[:], misc[0:1, 4:100], rr[:], start=True, stop=True)
                wN = wk.tile([E, BS], bf16, tag="wN")
                nc.vector.tensor_mul(wN[:], ex[:], rbc[:])

                # --- wsT [128,4,8]: enc[b] stationary x softmax column ---
                wsP = pss.tile([128, 4, BS], f32, tag="s")
                for c in range(4):
                    for b in range(BS):
                        nc.tensor.matmul(
                            wsP[:, c, b:b + 1], encB[:, b, c, :],
                            wN[:, b:b + 1], start=True, stop=True)
                nc.vector.tensor_copy(S0[:, 4:8, :], wsP[:])

                # --- L0: ws-dependent gate parts, then math ---
                gates_post0(pw0)
                gru_math(ph0, pw0, S0, "0")

                # --- L1: h0'-dependent gate parts, then math ---
                gates_post1(pw1)
                gru_math(ph1, pw1, S1, "1")

                # --- out projection (transposed): [T, 8] ---
                po = pss.tile([T, BS], f32, tag="s")
                for j in range(9):
                    mvo = S1[:, j, :] if j < 4 else S0[:, j, :]
                    nc.tensor.matmul(po[:], woT[:, j, :], mvo,
                                     start=(j == 0), stop=(j == 8))
                nc.vector.tensor_copy(outTb[:, t, :], po[:])

                # --- cur update for next step ---
                if t < DEC - 1:
                    nc.gpsimd.tensor_copy(S0[0:F, 8, :], inT[:, t, :])
                    nc.gpsimd.tensor_copy(S0[0:T, 8, :], outTb[:, t, :])

            # --- final: transpose [T,(t b)] -> [(t b),T], DMA out ---
            pfin = pss.tile([DEC * BS, T], f32, tag="s")
            nc.tensor.transpose(
                pfin[:], outTb[:].rearrange("T t b -> T (t b)"), ident4[:])
            osb = wk.tile([DEC * BS, T], f32, tag="osb")
            nc.vector.tensor_copy(osb[:], pfin[:])
            nc.sync.dma_start(out_e[:].rearrange("t b T -> (t b) T"), osb[:])

    # --- post-pass: walrus rejects instructions with more than a couple of
    # sync waits ("Too many sync wait commands").  Cap every instruction at
    # one wait by hoisting extras onto same-engine NoOps inserted just
    # before it (engine queues are in-order, so waiting earlier is safe). ---
    from concourse import mybir
    ctr = 0
    f = nc.m.functions[0]
    for blk in f.blocks:
        il = blk.instructions
        i = 0
        while i < len(il):
            inst = il[i]
            si = inst.sync_info
            waits = list(si.on_wait) if si is not None and si.on_wait else []
            if len(waits) > 1:
                SyncInfo = type(si)
                inst.sync_info = SyncInfo(
                    on_wait=[waits[-1]], on_update=list(si.on_update or []))
                for w in waits[:-1]:
                    nop = mybir.InstNoOp(name=f"I-nopw-{ctr}")
                    ctr += 1
                    nop.engine = inst.engine
                    nop.sync_info = SyncInfo(on_wait=[w], on_update=[])
                    nc.register_instruction(nop)
                    il.insert(i, nop)
                    i += 1
            i += 1

    return nc


def _prep_inputs(inputs, hidden, enc_outputs, target_indices,
                 W_attn, b_attn, v_attn,
                 gru_Wi0, gru_Wh0, gru_bi0, gru_bh0,
                 gru_Wi1, gru_Wh1, gru_bi1, gru_bh1,
                 W_out, b_out):
    """Build per-core input maps (host-side layout prep only)."""
    ti = np.asarray(target_indices)
    assert np.array_equal(ti, np.arange(T)), \
        "kernel specialized for target_indices == arange(T)"

    Wa_h = np.asarray(W_attn, np.float32)[:, :H]
    Wa_e = np.asarray(W_attn, np.float32)[:, H:]
    b_attn = np.asarray(b_attn, np.float32)
    v_attn = np.asarray(v_attn, np.float32)
    Wi0 = np.asarray(gru_Wi0, np.float32); Wh0 = np.asarray(gru_Wh0, np.float32)
    bi0 = np.asarray(gru_bi0, np.float32); bh0 = np.asarray(gru_bh0, np.float32)
    Wi1 = np.asarray(gru_Wi1, np.float32); Wh1 = np.asarray(gru_Wh1, np.float32)
    bi1 = np.asarray(gru_bi1, np.float32); bh1 = np.asarray(gru_bh1, np.float32)
    W_out = np.asarray(W_out, np.float32); b_out = np.asarray(b_out, np.float32)

    waeT = _bf16(Wa_e.T.reshape(4, 128, H).transpose(1, 0, 2))
    wahT = _bf16(Wa_h.T.reshape(4, 128, H).transpose(1, 0, 2))
    misc = np.zeros((128, 616), np.float32)
    misc[:, 0:4] = v_attn.reshape(4, 128).T
    misc[:, 4:100] = 1.0
    misc[0, 104:616] = b_attn
    misc = _bf16(misc)
    ident4 = _f32(np.eye(T, dtype=np.float32))

    # --- L0 gate weight blocks: z-order [h0(512) | ws(512) | cur(32),
    # one(@1056)]; stationary block (j,c) = Z[128j:128j+128, 128c:128c+128]
    Z0R = np.zeros((1152, H), np.float32)
    Z0R[0:512] = Wh0[0:512].T
    Z0R[512:1024] = Wi0[0:512, F:].T
    Z0R[1024:1056] = Wi0[0:512, 0:F].T
    Z0R[1056] = bi0[0:512] + bh0[0:512]
    Z0Z = np.zeros((1152, H), np.float32)
    Z0Z[0:512] = Wh0[512:1024].T
    Z0Z[512:1024] = Wi0[512:1024, F:].T
    Z0Z[1024:1056] = Wi0[512:1024, 0:F].T
    Z0Z[1056] = bi0[512:1024] + bh0[512:1024]
    Z0U = np.zeros((1152, H), np.float32)
    Z0U[0:512] = 0.5 * Wh0[1024:1536].T
    Z0U[512:1024] = Wi0[1024:1536, F:].T
    Z0U[1024:1056] = Wi0[1024:1536, 0:F].T
    Z0U[1056] = bi0[1024:1536] + 0.5 * bh0[1024:1536]
    Z0V = np.zeros((1152, H), np.float32)
    Z0V[0:512] = 0.5 * Wh0[1024:1536].T
    Z0V[1056] = 0.5 * bh0[1024:1536]

    # --- L1: z-order [h0'(512) | h1(512) | one(@1024)] ---
    Z1R = np.zeros((1152, H), np.float32)
    Z1R[0:512] = Wi1[0:512].T
    Z1R[512:1024] = Wh1[0:512].T
    Z1R[1024] = bi1[0:512] + bh1[0:512]
    Z1Z = np.zeros((1152, H), np.float32)
    Z1Z[0:512] = Wi1[512:1024].T
    Z1Z[512:1024] = Wh1[512:1024].T
    Z1Z[1024] = bi1[512:1024] + bh1[512:1024]
    Z1U = np.zeros((1152, H), np.float32)
    Z1U[0:512] = Wi1[1024:1536].T
    Z1U[512:1024] = 0.5 * Wh1[1024:1536].T
    Z1U[1024] = bi1[1024:1536] + 0.5 * bh1[1024:1536]
    Z1V = np.zeros((1152, H), np.float32)
    Z1V[512:1024] = 0.5 * Wh1[1024:1536].T
    Z1V[1024] = 0.5 * bh1[1024:1536]

    def blocks(mats, js):
        blks = []
        for Zm in mats:
            for c in range(4):
                for j in js:
                    blks.append(Zm[128 * j:128 * j + 128,
                                   128 * c:128 * c + 128])
        return _bf16(np.stack(blks).transpose(1, 0, 2))

    wL0 = blocks((Z0R, Z0Z, Z0U), range(9))
    wL1 = blocks((Z1R, Z1Z, Z1U), range(8))
    bias3 = np.zeros((1, 20, 128), np.float32)
    bias3[0, 0:4] = Z0V[1056].reshape(4, 128)
    bias3[0, 4:16] = np.stack([Zm[1024, 128 * c:128 * c + 128]
                               for Zm in (Z1R, Z1Z, Z1U) for c in range(4)])
    bias3[0, 16:20] = Z1V[1024].reshape(4, 128)
    bias3 = _bf16(bias3)

    # --- out projection: z-order [h1'(512) | ws(512) | cur(32), one(@1056)]
    ZO = np.zeros((1152, T), np.float32)
    ZO[0:512] = W_out[:, 0:H].T
    ZO[512:1024] = W_out[:, H:2 * H].T
    ZO[1024:1056] = W_out[:, 2 * H:2 * H + F].T
    ZO[1056] = b_out
    woT = _bf16(ZO.reshape(9, 128, T).transpose(1, 0, 2))

    inputs = np.asarray(inputs, np.float32)
    hidden = np.asarray(hidden, np.float32)
    enc_outputs = np.asarray(enc_outputs, np.float32)

    in_maps = []
    for cc in range(N_CORES):
        s = slice(cc * BS, (cc + 1) * BS)
        encc = enc_outputs[s]                      # [8, 96, 512]
        encT = _bf16(encc.transpose(1, 0, 2).reshape(E * BS, H).T
                     .reshape(4, 128, E * BS).transpose(1, 0, 2))
        encB = _bf16(encc.transpose(1, 0, 2).reshape(E, BS, 4, 128))
        h0 = hidden[0, s]                          # [8, 512]
        h1 = hidden[1, s]
        s0init = np.zeros((128, 9, BS), np.float32)
        s0init[:, 0:4, :] = h0.T.reshape(4, 128, BS).transpose(1, 0, 2)
        s0init[0:F, 8, :] = inputs[s, 0, :].T
        s0init[F, 8, :] = 1.0
        s1init = np.zeros((128, 5, BS), np.float32)
        s1init[:, 0:4, :] = h1.T.reshape(4, 128, BS).transpose(1, 0, 2)
        s1init[0, 4, :] = 1.0
        in_maps.append({
            "s0init": _bf16(s0init), "s1init": _bf16(s1init),
            "inT": _bf16(inputs[s].transpose(2, 1, 0)),
            "encT": encT, "encB": encB,
            "waeT": waeT, "wahT": wahT, "misc": misc,
            "wL0": wL0, "wL1": wL1, "woT": woT, "ident4": ident4,
            "bias3": bias3,
        })
    return in_maps


def get_nc():
    if "nc" not in _COMPILED:
        _COMPILED["nc"] = build_nc()
    return _COMPILED["nc"]


def kernel(**inputs):
    from concourse.bass_utils import run_bass_kernel_spmd
    nc = get_nc()
    in_maps = _prep_inputs(**inputs)
    res = run_bass_kernel_spmd(nc, in_maps, list(range(N_CORES)))
    out = np.concatenate([res.results[c]["out"].transpose(1, 0, 2)
                          for c in range(N_CORES)], axis=0)
    return np.ascontiguousarray(out, dtype=np.float32)
